# revision 16
# baseline (speedup 1.0000x reference)
"""Trainium2 Bass kernel for the MFVI second-order CRF message-passing module.

Math (per batch element, per iteration):
    q_sm = softmax(q, axis=-1)                               # over T=256
    msg[s] = q_sm[s-1]@T1 + q_sm[s-2]@T2 + q_sm[s+1]@T1' + q_sm[s+2]@T2'
    q    = (unary + msg + start/end-corrections) * mask

Strategy (v2) - 8 cores, data-parallel over batch (4 elems/core):
  * Length-aware column skipping: batch elems are sorted by length and
    dealt round-robin into BPC=4 "slots" so every core's slot-n elem has a
    similar length; the program only computes columns [0, C_n) where
    C_n = max length in slot n (~3.4k cols/core vs 4k full).  Tail senders
    beyond each elem's true length L are uniform-softmax constants; their
    message contributions are folded into the corrected unary on the host
    and the on-device softmax columns [L, C_n) are zeroed via a masked
    reciprocal, so junk there never propagates.
  * Everything is scaled by 64 on host (u*64) so the softmax operand can be
    quantized to fp8e4 with all values in the normal range; exp() uses
    scale=1/64 to undo it, and the colsum "ones" matrix holds 1/64 so the
    DVE reciprocal directly yields rb = 64/z.
  * Conv matmuls run in fp8e4 DoubleRow mode: both 128-row chunks of the
    T=256 contraction are processed in a single pass at 0.5 cycles/column
    (4x the fp32r rate).  Weights [128, 2, 128] per (tap, out-chunk).
  * The corrected-unary add is done ON THE PE via an fp16 identity matmul
    that joins each PSUM accumulation group (start=True), so the Act engine
    can read exp() straight out of PSUM and DVE only does recip + half the
    normalize (other half on GpSimd).  Final iteration skips the identity
    and lets DVE fuse the add into the PSUM evacuation.
  * Each elem is split into two column halves ("velems") so conv PSUM
    tiles stay <= 2 banks; 24 pipeline steps with a 2-round software
    lookahead keep the PE queue saturated.
  * All HBM I/O is fp16 (64*unary, 64*ucorr in; 64*q out, /64 on host).
"""
import os
import sys

sys.path.insert(0, "/opt/trn_rl_repo")

import numpy as np
import ml_dtypes

import concourse.mybir as mybir
from concourse.bass import Bass
from concourse.tile import TileContext
from concourse import bass_utils

B, S, T = 32, 1024, 256
WINDOW = 2
ITERS = 3
N_CORES = 8
BPC = B // N_CORES          # batch elems per core (= slots)
NCH = T // 128              # partition chunks of T
SCALE = 64.0

NODR = bool(os.environ.get("MFVI_NODR"))    # fallback: bf16 conv, no DoubleRow
NOGP = bool(os.environ.get("MFVI_NOGP"))    # fallback: normalize fully on DVE
WARMUP = 6

# taps: (mat index, shift) ; mats = [T1, T2, T1^T, T2^T]
SHIFTS = [(0, -1), (1, -2), (2, +1), (3, +2)]


def _split_sync_waits(nc):
    """walrus accepts at most ONE sync wait per instruction; Tile emits
    several. Move extras onto same-engine NoOps."""
    ctr = 0
    for f in nc.m.functions:
        for block in f.blocks:
            out = []
            changed = False
            for inst in block.instructions:
                si = inst.sync_info
                waits = list(si.on_wait) if si is not None and si.on_wait else []
                if len(waits) > 1:
                    changed = True
                    for w in waits[:-1]:
                        ctr += 1
                        nop = mybir.InstNoOp(
                            name=f"I-waitsplit-{ctr}",
                            engine=inst.engine, ins=[], outs=[])
                        nop.sync_info = mybir.SyncInfo(on_wait=[w], on_update=[])
                        out.append(nop)
                    si.on_wait = [waits[-1]]
                    inst.sync_info = si
                out.append(inst)
            if changed:
                block.instructions = out
    return nc


def _plan(lengths):
    L = np.asarray(lengths, np.int64)
    order = np.argsort(-L, kind="stable")
    slots = [order[N_CORES * n:N_CORES * (n + 1)] for n in range(BPC)]
    C, F = [], []
    for n in range(BPC):
        ls = L[slots[n]]
        C.append(min(S, int(-(-int(ls.max()) // 16) * 16)))
        F.append(int(ls.min()))
    Ch = [c // 2 for c in C]
    M = [C[n] - F[n] for n in range(BPC)]
    return dict(order=order, slots=slots, C=C, F=F, Ch=Ch, M=M, L=L)


def _build(C, F, Ch, M):
    f32 = mybir.dt.float32
    f16 = mybir.dt.float16
    bf16 = mybir.dt.bfloat16
    f8 = mybir.dt.float8e4
    qdt = bf16 if NODR else f8
    DR = None if NODR else mybir.MatmulPerfMode.DoubleRow

    nc = Bass(trn_type="TRN2", target_bir_lowering=False, debug=False,
              num_devices=N_CORES)

    u_d = [nc.dram_tensor(f"u{n}", [128, NCH * C[n]], f16,
                          kind="ExternalInput").ap() for n in range(BPC)]
    uc_d = [nc.dram_tensor(f"uc{n}", [128, NCH * C[n]], f16,
                           kind="ExternalInput").ap() for n in range(BPC)]
    q_d = [nc.dram_tensor(f"q{n}", [128, NCH * C[n]], f16,
                          kind="ExternalOutput").ap() for n in range(BPC)]
    # DR: 8 blocks of [128, 2, 128] fp8 ; NODR: 16 blocks of [128, 128] bf16
    w_d = nc.dram_tensor("wdr", [128, 2048], qdt, kind="ExternalInput").ap()
    id_d = nc.dram_tensor("ident", [128, 128], f16, kind="ExternalInput").ap()
    on_d = nc.dram_tensor("onesb", [128, 128], bf16, kind="ExternalInput").ap()
    MSUM = sum(M)
    mk_d = nc.dram_tensor("mkall", [128, max(MSUM, 1)], f32,
                          kind="ExternalInput").ap()

    with TileContext(nc) as tc:
        with tc.tile_pool(name="persist", bufs=1) as pp, \
             tc.tile_pool(name="ebfp", bufs=6) as ebfp, \
             tc.tile_pool(name="rbp", bufs=6) as rbp, \
             tc.tile_pool(name="psum", bufs=3, space="PSUM") as psp:

            u_t = [pp.tile([128, NCH * C[n]], f16, tag=f"u{n}", name=f"u{n}")
                   for n in range(BPC)]
            uc_t = [pp.tile([128, NCH * C[n]], f16, tag=f"uc{n}", name=f"uc{n}")
                    for n in range(BPC)]
            qf_t = [pp.tile([128, NCH * C[n]], f16, tag=f"qf{n}", name=f"qf{n}")
                    for n in range(BPC)]
            qs_t = [pp.tile([128, NCH * (C[n] + 2 * WINDOW)], qdt,
                            tag=f"qs{n}", name=f"qs{n}") for n in range(BPC)]
            w_t = pp.tile([128, 2048], qdt, tag="w", name="w")
            id_t = pp.tile([128, 128], f16, tag="id", name="id")
            on_t = pp.tile([128, 128], bf16, tag="on", name="on")
            mk_t = pp.tile([128, max(MSUM, 1)], f32, tag="mk", name="mk")
            mk_off = [sum(M[:n]) for n in range(BPC)]

            # --- input DMA: elem-0 unary halves first (exp(0) gates the
            # pipeline fill), then the small constants, then the rest. ---
            def dma_half(dst, src, n, h):
                o, w = (0, Ch[n]) if h == 0 else (Ch[n], C[n] - Ch[n])
                cn = C[n]
                nc.sync.dma_start(
                    out=dst[:, 0:NCH * cn].rearrange(
                        "p (a b) -> p a b", a=NCH)[:, :, o:o + w],
                    in_=src[:, 0:NCH * cn].rearrange(
                        "p (a b) -> p a b", a=NCH)[:, :, o:o + w])

            def dma_in(n):
                nc.sync.dma_start(out=u_t[n][:], in_=u_d[n][:])
                nc.sync.dma_start(out=uc_t[n][:], in_=uc_d[n][:])

            nc.sync.dma_start(out=on_t, in_=on_d)
            dma_half(u_t[0], u_d[0], 0, 0)
            dma_half(u_t[0], u_d[0], 0, 1)
            nc.sync.dma_start(out=w_t, in_=w_d)
            nc.sync.dma_start(out=id_t, in_=id_d)
            dma_half(uc_t[0], uc_d[0], 0, 0)
            dma_half(uc_t[0], uc_d[0], 0, 1)
            if MSUM > 0:
                nc.sync.dma_start(out=mk_t, in_=mk_d)
            dma_in(1)

            # qsm guard columns (2 cols each side of each chunk) -> 0
            for n in range(BPC):
                row = C[n] + 2 * WINDOW
                for c in range(NCH):
                    nc.gpsimd.memset(qs_t[n][:, c * row:c * row + WINDOW], 0.0)
                    nc.gpsimd.memset(
                        qs_t[n][:, c * row + WINDOW + C[n]:(c + 1) * row], 0.0)

            # PE p-state warmup (~3us of dummy matmuls)
            pwarm = psp.tile([128, 512], f32, tag="z", name="pwarm", bufs=2)
            for k in range(WARMUP):
                nc.tensor.matmul(pwarm[:, 0:128], on_t[:], on_t[:],
                                 start=True, stop=True)

            # ---------------- pipeline ----------------
            # Steps interleave TWO elems per iteration (lanes e0A,e0B,e1A,e1B)
            # so the per-velem softmax latency chain (conv -> exp -> colsum ->
            # recip -> norm -> conv, ~3.5us) overlaps with ~4 PE rounds of
            # other lanes' work.  exp(x) consumes the conv PSUM of x-4.
            NSTEP = BPC * ITERS * 2
            STEPS = []
            for pair in range(BPC // 2):
                for it in range(ITERS):
                    for e in (2 * pair, 2 * pair + 1):
                        for h in range(2):
                            STEPS.append((e, it, h))
            psums = {}

            def sih(x):
                return STEPS[x]

            def emit_exp(x):
                n, it, h = sih(x)
                ch, cn = Ch[n], C[n]
                ebf = ebfp.tile([128, 1024], bf16, tag="ebf", name=f"ebf{x}")
                dst = ebf[:, 0:NCH * ch].rearrange("p (a b) -> p a b", a=NCH)
                if it == 0:
                    src = u_t[n][:, 0:NCH * cn].rearrange(
                        "p (a b) -> p a b", a=NCH)[:, :, h * ch:(h + 1) * ch]
                else:
                    pv = psums[x - 4]
                    src = pv[:, 0:1024].rearrange(
                        "p (a b) -> p a b", a=NCH)[:, :, 0:ch]
                nc.scalar.activation(out=dst, in_=src,
                                     func=mybir.ActivationFunctionType.Exp,
                                     scale=1.0 / SCALE)
                return ebf

            ebfs = {}

            def emit_zchain(x):
                n, it, h = sih(x)
                ch, cn = Ch[n], C[n]
                ebf = ebfs[x]
                zt = psp.tile([128, 512], f32, tag="z", name=f"z{x}", bufs=2)
                for c in range(NCH):
                    nc.tensor.matmul(zt[:, 0:ch], on_t[:],
                                     ebf[:, c * ch:(c + 1) * ch],
                                     start=(c == 0), stop=(c == NCH - 1))
                rb = rbp.tile([128, 512], f32, tag="rb", name=f"rb{x}")
                nc.vector.reciprocal(rb[:, 0:ch], zt[:, 0:ch])
                row = cn + 2 * WINDOW

                def norm(c, o, w, eng):
                    dst = qs_t[n][:, c * row + WINDOW + h * ch + o:
                                  c * row + WINDOW + h * ch + o + w]
                    if eng is nc.gpsimd and not NOGP:
                        eng.scalar_tensor_tensor(
                            out=dst, in0=ebf[:, c * ch + o:c * ch + o + w],
                            scalar=1.0, in1=rb[:, o:o + w],
                            op0=mybir.AluOpType.mult,
                            op1=mybir.AluOpType.mult)
                    else:
                        nc.vector.tensor_mul(
                            out=dst, in0=ebf[:, c * ch + o:c * ch + o + w],
                            in1=rb[:, o:o + w])

                if h == 1 and M[n] > 0:
                    # junk columns [lo, ch) need the masked reciprocal; the
                    # clean bulk [0, lo) only needs recip and goes straight
                    # to the Pool so the mask never gates it.
                    lo = F[n] - ch
                    nc.vector.tensor_mul(
                        out=rb[:, lo:lo + M[n]], in0=rb[:, lo:lo + M[n]],
                        in1=mk_t[:, mk_off[n]:mk_off[n] + M[n]])
                    norm(0, 0, lo, nc.gpsimd)
                    norm(1, 0, lo, nc.gpsimd)
                    norm(0, lo, M[n], nc.vector)
                    norm(1, lo, M[n], nc.vector)
                else:
                    sp = (2 * (ch // 5)) // 8 * 8
                    norm(0, 0, sp, nc.vector)
                    norm(0, sp, ch - sp, nc.gpsimd)
                    norm(1, 0, ch, nc.gpsimd)

            def emit_conv(x):
                n, it, h = sih(x)
                ch, cn = Ch[n], C[n]
                row = cn + 2 * WINDOW
                pv = psp.tile([128, 1024], f32, tag="cv", name=f"cv{x}")
                psums[x] = pv
                qsr = qs_t[n][:, 0:NCH * row].rearrange("p (a b) -> p a b",
                                                        a=NCH)
                ucr = uc_t[n][:, 0:NCH * cn].rearrange("p (a b) -> p a b",
                                                       a=NCH)
                half = ch // 2
                blocks = [(0, half), (half, ch - half)] if not NODR \
                    else [(0, ch)]
                base = WINDOW + h * ch
                for mc in range(NCH):
                    for (co, nn) in blocks:
                        out = pv[:, mc * 512 + co:mc * 512 + co + nn]
                        first = False
                        o = mc * cn + h * ch + co
                        nc.tensor.matmul(
                            out, id_t[:], uc_t[n][:, o:o + nn],
                            start=True, stop=False)
                        if NODR:
                            nmm = len(SHIFTS) * NCH
                            cnt = 0
                            for (t, d) in SHIFTS:
                                for kt in range(NCH):
                                    lhs = w_t[:, ((t * 2 + mc) * 2 + kt) * 128:
                                              ((t * 2 + mc) * 2 + kt + 1) * 128]
                                    rhs = qs_t[n][:, kt * row + base + co + d:
                                                  kt * row + base + co + d + nn]
                                    nc.tensor.matmul(
                                        out, lhs, rhs,
                                        start=(first and cnt == 0),
                                        stop=(cnt == nmm - 1))
                                    cnt += 1
                        else:
                            for ti, (t, d) in enumerate(SHIFTS):
                                lhs = w_t[:, (t * 2 + mc) * 256:
                                          (t * 2 + mc + 1) * 256].rearrange(
                                    "p (a b) -> p a b", a=2)
                                rhs = qsr[:, :, base + co + d:
                                          base + co + d + nn]
                                nc.tensor.matmul(
                                    out, lhs, rhs,
                                    start=(first and ti == 0),
                                    stop=(ti == len(SHIFTS) - 1),
                                    perf_mode=mybir.MatmulPerfMode.DoubleRow)
            def emit_out(x):
                n, it, h = sih(x)
                if it != ITERS - 1:
                    return
                ch, cn = Ch[n], C[n]
                pv = psums[x]
                # PSUM already holds 64*(msg+ucorr) thanks to the ident
                # matmul; evacuate on the Act engine and ship.
                if True:
                    qfw = qf_t[n][:, 0:NCH * cn].rearrange(
                        "p (a b) -> p a b", a=NCH)[:, :, h * ch:(h + 1) * ch]
                    pvr = pv[:, 0:1024].rearrange(
                        "p (a b) -> p a b", a=NCH)[:, :, 0:ch]
                    nc.scalar.activation(
                        out=qfw, in_=pvr,
                        func=mybir.ActivationFunctionType.Copy)
                    qfr = qf_t[n][:, 0:NCH * cn].rearrange(
                        "p (a b) -> p a b", a=NCH)[:, :, h * ch:(h + 1) * ch]
                    qdr = q_d[n][:, 0:NCH * cn].rearrange(
                        "p (a b) -> p a b", a=NCH)[:, :, h * ch:(h + 1) * ch]
                    nc.sync.dma_start(out=qdr, in_=qfr)

            # round r emits: zchain(r), conv(r-2), exp(r+2), out(r-2).
            # (the final-iteration PSUM evacuation copy is emitted AFTER
            # exp(r+2) so it never delays the next exp in the Act queue)
            # PE queue per round = [colsum(r), conv(r-2)]: conv trails its
            # own norm by 2 rounds and the seam-partner norm by 1 round, so
            # the PE never waits; exp(r+2) is emitted right after conv(r-2)
            # (whose PSUM it reads), keeping <=3 conv PSUMs alive.
            ebfs[0] = emit_exp(0)
            ebfs[1] = emit_exp(1)
            for r in range(NSTEP + 2):
                if r == 0:
                    dma_in(2)
                if r == 2:
                    dma_in(3)
                if r < NSTEP:
                    emit_zchain(r)
                if 0 <= r - 2 < NSTEP:
                    emit_conv(r - 2)
                if r + 2 < NSTEP:
                    ebfs[r + 2] = emit_exp(r + 2)
                if 0 <= r - 2 < NSTEP:
                    emit_out(r - 2)

    _split_sync_waits(nc)
    return nc


_CACHE = {}
_LAST_NC = None


def _get_nc(plan=None):
    global _LAST_NC
    if plan is None:
        return _LAST_NC
    key = (tuple(plan["C"]), tuple(plan["F"]))
    if key not in _CACHE:
        _CACHE[key] = _build(plan["C"], plan["F"], plan["Ch"], plan["M"])
    _LAST_NC = _CACHE[key]
    return _LAST_NC


def _host_prep(unary_score, mask, transitions, start_transitions,
               end_transitions, lengths, plan):
    f16 = np.float16
    L = plan["L"]
    unary = (np.asarray(unary_score, np.float32)
             * np.asarray(mask, np.float32)[..., None])      # [B,S,T]
    trans = np.asarray(transitions, np.float32)
    ucorr = unary.copy()
    ucorr[:, 0:WINDOW, :] += np.asarray(start_transitions, np.float32)
    rowmean = [trans[j].sum(axis=1) / T for j in range(WINDOW)]  # u @ Tj^T
    endt = np.asarray(end_transitions, np.float32)
    for b in range(B):
        lb = int(L[b])
        for j in range(1, WINDOW + 1):
            ucorr[b, lb - j, :] += endt[j - 1]
        # uniform-softmax senders in the masked tail [lb, S)
        if lb <= S - 1:
            ucorr[b, lb - 1, :] += rowmean[0]      # sender lb,   j=1
            ucorr[b, lb - 2, :] += rowmean[1]      # sender lb,   j=2
        if lb <= S - 2:
            ucorr[b, lb - 1, :] += rowmean[1]      # sender lb+1, j=2
    mats = [trans[0], trans[1], trans[0].T, trans[1].T]

    qdt = ml_dtypes.bfloat16 if NODR else ml_dtypes.float8_e4m3
    wpk = np.zeros((128, 2048), np.float32)
    for t in range(4):
        for mc in range(NCH):
            for kt in range(NCH):
                blk = mats[t][kt * 128:(kt + 1) * 128,
                              mc * 128:(mc + 1) * 128]       # [kp, m]
                if NODR:
                    o = ((t * 2 + mc) * 2 + kt) * 128
                    wpk[:, o:o + 128] = blk
                else:
                    o = (t * 2 + mc) * 256 + kt * 128
                    wpk[:, o:o + 128] = blk
    wpk = wpk.astype(qdt)
    ident = np.eye(128, dtype=f16)
    onesb = np.full((128, 128), 1.0 / SCALE, ml_dtypes.bfloat16)

    u64 = (unary * SCALE).astype(f16)
    uc64 = (ucorr * SCALE).astype(f16)

    in_maps = []
    for core in range(N_CORES):
        m = {"wdr": wpk, "ident": ident, "onesb": onesb}
        for n in range(BPC):
            b = int(plan["slots"][n][core])
            cn = plan["C"][n]
            m[f"u{n}"] = np.ascontiguousarray(
                u64[b, :cn].T.reshape(NCH, 128, cn))
            m[f"uc{n}"] = np.ascontiguousarray(
                uc64[b, :cn].T.reshape(NCH, 128, cn))
            if plan["M"][n] > 0:
                lb = int(L[b])
                col = np.arange(plan["F"][n], cn) < lb
                m[f"mk{n}"] = np.ascontiguousarray(
                    np.broadcast_to(col[None, :].astype(np.float32),
                                    (128, plan["M"][n])))
        in_maps.append(m)
    return in_maps


def kernel(token_feats, unary_score, mask, transitions, start_transitions,
           end_transitions, lengths):
    plan = _plan(lengths)
    nc = _get_nc(plan)
    in_maps = _host_prep(unary_score, mask, transitions, start_transitions,
                         end_transitions, lengths, plan)
    res = bass_utils.run_bass_kernel_spmd(nc, in_maps,
                                          core_ids=list(range(N_CORES)))
    out = np.zeros((B, S, T), np.float32)
    L = plan["L"]
    for core in range(N_CORES):
        for n in range(BPC):
            b = int(plan["slots"][n][core])
            cn = plan["C"][n]
            qv = np.asarray(res.results[core][f"q{n}"],
                            np.float32)                       # [2,128,cn]
            lb = int(L[b])
            out[b, :lb, :] = qv.reshape(T, cn).T[:lb] / SCALE
    return out


# revision 17
# speedup vs baseline: 1.0424x; 1.0424x over previous
"""Trainium2 Bass kernel for the MFVI second-order CRF message-passing module.

Math (per batch element, per iteration):
    q_sm = softmax(q, axis=-1)                               # over T=256
    msg[s] = q_sm[s-1]@T1 + q_sm[s-2]@T2 + q_sm[s+1]@T1' + q_sm[s+2]@T2'
    q    = (unary + msg + start/end-corrections) * mask

Strategy (v2) - 8 cores, data-parallel over batch (4 elems/core):
  * Length-aware column skipping: batch elems are sorted by length and
    dealt round-robin into BPC=4 "slots" so every core's slot-n elem has a
    similar length; the program only computes columns [0, C_n) where
    C_n = max length in slot n (~3.4k cols/core vs 4k full).  Tail senders
    beyond each elem's true length L are uniform-softmax constants; their
    message contributions are folded into the corrected unary on the host
    and the on-device softmax columns [L, C_n) are zeroed via a masked
    reciprocal, so junk there never propagates.
  * Everything is scaled by 64 on host (u*64) so the softmax operand can be
    quantized to fp8e4 with all values in the normal range; exp() uses
    scale=1/64 to undo it, and the colsum "ones" matrix holds 1/64 so the
    DVE reciprocal directly yields rb = 64/z.
  * Conv matmuls run in fp8e4 DoubleRow mode: both 128-row chunks of the
    T=256 contraction are processed in a single pass at 0.5 cycles/column
    (4x the fp32r rate).  Weights [128, 2, 128] per (tap, out-chunk).
  * The corrected-unary add is done ON THE PE via an fp16 identity matmul
    that joins each PSUM accumulation group (start=True), so the Act engine
    can read exp() straight out of PSUM and DVE only does recip + half the
    normalize (other half on GpSimd).  Final iteration skips the identity
    and lets DVE fuse the add into the PSUM evacuation.
  * Each elem is split into two column halves ("velems") so conv PSUM
    tiles stay <= 2 banks; 24 pipeline steps with a 2-round software
    lookahead keep the PE queue saturated.
  * All HBM I/O is fp16 (64*unary, 64*ucorr in; 64*q out, /64 on host).
"""
import os
import sys

sys.path.insert(0, "/opt/trn_rl_repo")

import numpy as np
import ml_dtypes

import concourse.mybir as mybir
from concourse.bass import Bass
from concourse.tile import TileContext
from concourse import bass_utils

B, S, T = 32, 1024, 256
WINDOW = 2
ITERS = 3
N_CORES = 8
BPC = B // N_CORES          # batch elems per core (= slots)
NCH = T // 128              # partition chunks of T
SCALE = 64.0

NODR = bool(os.environ.get("MFVI_NODR"))    # fallback: bf16 conv, no DoubleRow
NOGP = bool(os.environ.get("MFVI_NOGP"))    # fallback: normalize fully on DVE
WARMUP = 6

# taps: (mat index, shift) ; mats = [T1, T2, T1^T, T2^T]
SHIFTS = [(0, -1), (1, -2), (2, +1), (3, +2)]


def _split_sync_waits(nc):
    """walrus accepts at most ONE sync wait per instruction; Tile emits
    several. Move extras onto same-engine NoOps."""
    ctr = 0
    for f in nc.m.functions:
        for block in f.blocks:
            out = []
            changed = False
            for inst in block.instructions:
                si = inst.sync_info
                waits = list(si.on_wait) if si is not None and si.on_wait else []
                if len(waits) > 1:
                    changed = True
                    for w in waits[:-1]:
                        ctr += 1
                        nop = mybir.InstNoOp(
                            name=f"I-waitsplit-{ctr}",
                            engine=inst.engine, ins=[], outs=[])
                        nop.sync_info = mybir.SyncInfo(on_wait=[w], on_update=[])
                        out.append(nop)
                    si.on_wait = [waits[-1]]
                    inst.sync_info = si
                out.append(inst)
            if changed:
                block.instructions = out
    return nc


def _plan(lengths):
    L = np.asarray(lengths, np.int64)
    order = np.argsort(-L, kind="stable")
    slots = [order[N_CORES * n:N_CORES * (n + 1)] for n in range(BPC)]
    C, F = [], []
    for n in range(BPC):
        ls = L[slots[n]]
        C.append(min(S, int(-(-int(ls.max()) // 16) * 16)))
        F.append(int(ls.min()))
    Ch = [c // 2 for c in C]
    M = [C[n] - F[n] for n in range(BPC)]
    return dict(order=order, slots=slots, C=C, F=F, Ch=Ch, M=M, L=L)


def _build(C, F, Ch, M):
    f32 = mybir.dt.float32
    f16 = mybir.dt.float16
    bf16 = mybir.dt.bfloat16
    f8 = mybir.dt.float8e4
    qdt = bf16 if NODR else f8
    DR = None if NODR else mybir.MatmulPerfMode.DoubleRow

    nc = Bass(trn_type="TRN2", target_bir_lowering=False, debug=False,
              num_devices=N_CORES)

    u_d = [nc.dram_tensor(f"u{n}", [128, NCH * C[n]], f16,
                          kind="ExternalInput").ap() for n in range(BPC)]
    uc_d = [nc.dram_tensor(f"uc{n}", [128, NCH * C[n]], f16,
                           kind="ExternalInput").ap() for n in range(BPC)]
    q_d = [nc.dram_tensor(f"q{n}", [128, NCH * C[n]], f16,
                          kind="ExternalOutput").ap() for n in range(BPC)]
    # DR: 8 blocks of [128, 2, 128] fp8 ; NODR: 16 blocks of [128, 128] bf16
    w_d = nc.dram_tensor("wdr", [128, 2048], qdt, kind="ExternalInput").ap()
    id_d = nc.dram_tensor("ident", [128, 128], f16, kind="ExternalInput").ap()
    on_d = nc.dram_tensor("onesb", [128, 128], bf16, kind="ExternalInput").ap()
    MSUM = sum(M)
    mk_d = nc.dram_tensor("mkall", [128, max(MSUM, 1)], f32,
                          kind="ExternalInput").ap()

    with TileContext(nc) as tc:
        with tc.tile_pool(name="persist", bufs=1) as pp, \
             tc.tile_pool(name="ebfp", bufs=6) as ebfp, \
             tc.tile_pool(name="rbp", bufs=6) as rbp, \
             tc.tile_pool(name="psum", bufs=3, space="PSUM") as psp:

            u_t = [pp.tile([128, NCH * C[n]], f16, tag=f"u{n}", name=f"u{n}")
                   for n in range(BPC)]
            uc_t = [pp.tile([128, NCH * C[n]], f16, tag=f"uc{n}", name=f"uc{n}")
                    for n in range(BPC)]
            qf_t = [pp.tile([128, NCH * C[n]], f16, tag=f"qf{n}", name=f"qf{n}")
                    for n in range(BPC)]
            qs_t = [pp.tile([128, NCH * (C[n] + 2 * WINDOW)], qdt,
                            tag=f"qs{n}", name=f"qs{n}") for n in range(BPC)]
            w_t = pp.tile([128, 2048], qdt, tag="w", name="w")
            id_t = pp.tile([128, 128], f16, tag="id", name="id")
            on_t = pp.tile([128, 128], bf16, tag="on", name="on")
            mk_t = pp.tile([128, max(MSUM, 1)], f32, tag="mk", name="mk")
            mk_off = [sum(M[:n]) for n in range(BPC)]

            # --- input DMA: elem-0 unary halves first (exp(0) gates the
            # pipeline fill), then the small constants, then the rest. ---
            def dma_half(dst, src, n, h):
                o, w = (0, Ch[n]) if h == 0 else (Ch[n], C[n] - Ch[n])
                cn = C[n]
                nc.sync.dma_start(
                    out=dst[:, 0:NCH * cn].rearrange(
                        "p (a b) -> p a b", a=NCH)[:, :, o:o + w],
                    in_=src[:, 0:NCH * cn].rearrange(
                        "p (a b) -> p a b", a=NCH)[:, :, o:o + w])

            def dma_in(n):
                nc.sync.dma_start(out=u_t[n][:], in_=u_d[n][:])
                nc.sync.dma_start(out=uc_t[n][:], in_=uc_d[n][:])

            # ordered by first use: exp(0) u0h0, exp(1) u0h1, exp(2) u1h0,
            # colsum ones, zchain(1) mask, conv(0) weights/ident/ucorr...
            nc.sync.dma_start(out=on_t, in_=on_d)
            dma_half(u_t[0], u_d[0], 0, 0)
            dma_half(u_t[0], u_d[0], 0, 1)
            dma_half(u_t[1], u_d[1], 1, 0)
            nc.sync.dma_start(out=w_t, in_=w_d)
            nc.sync.dma_start(out=id_t, in_=id_d)
            if MSUM > 0:
                nc.sync.dma_start(out=mk_t, in_=mk_d)
            dma_half(u_t[1], u_d[1], 1, 1)
            dma_half(uc_t[0], uc_d[0], 0, 0)
            dma_half(uc_t[0], uc_d[0], 0, 1)
            dma_half(uc_t[1], uc_d[1], 1, 0)
            dma_half(uc_t[1], uc_d[1], 1, 1)

            # qsm guard columns (2 cols each side of each chunk) -> 0
            for n in range(BPC):
                row = C[n] + 2 * WINDOW
                for c in range(NCH):
                    nc.gpsimd.memset(qs_t[n][:, c * row:c * row + WINDOW], 0.0)
                    nc.gpsimd.memset(
                        qs_t[n][:, c * row + WINDOW + C[n]:(c + 1) * row], 0.0)

            # PE p-state warmup (~3us of dummy matmuls)
            pwarm = psp.tile([128, 512], f32, tag="z", name="pwarm", bufs=2)
            for k in range(WARMUP):
                nc.tensor.matmul(pwarm[:, 0:128], on_t[:], on_t[:],
                                 start=True, stop=True)

            # ---------------- pipeline ----------------
            # Steps interleave TWO elems per iteration (lanes e0A,e0B,e1A,e1B)
            # so the per-velem softmax latency chain (conv -> exp -> colsum ->
            # recip -> norm -> conv, ~3.5us) overlaps with ~4 PE rounds of
            # other lanes' work.  exp(x) consumes the conv PSUM of x-4.
            NSTEP = BPC * ITERS * 2
            STEPS = []
            for pair in range(BPC // 2):
                for it in range(ITERS):
                    for e in (2 * pair, 2 * pair + 1):
                        for h in range(2):
                            STEPS.append((e, it, h))
            psums = {}

            def sih(x):
                return STEPS[x]

            def emit_exp(x):
                n, it, h = sih(x)
                ch, cn = Ch[n], C[n]
                ebf = ebfp.tile([128, 1024], bf16, tag="ebf", name=f"ebf{x}")
                dst = ebf[:, 0:NCH * ch].rearrange("p (a b) -> p a b", a=NCH)
                if it == 0:
                    src = u_t[n][:, 0:NCH * cn].rearrange(
                        "p (a b) -> p a b", a=NCH)[:, :, h * ch:(h + 1) * ch]
                else:
                    pv = psums[x - 4]
                    src = pv[:, 0:1024].rearrange(
                        "p (a b) -> p a b", a=NCH)[:, :, 0:ch]
                nc.scalar.activation(out=dst, in_=src,
                                     func=mybir.ActivationFunctionType.Exp,
                                     scale=1.0 / SCALE)
                return ebf

            ebfs = {}

            def emit_zchain(x):
                n, it, h = sih(x)
                ch, cn = Ch[n], C[n]
                ebf = ebfs[x]
                zt = psp.tile([128, 512], f32, tag="z", name=f"z{x}", bufs=2)
                for c in range(NCH):
                    nc.tensor.matmul(zt[:, 0:ch], on_t[:],
                                     ebf[:, c * ch:(c + 1) * ch],
                                     start=(c == 0), stop=(c == NCH - 1))
                rb = rbp.tile([128, 512], f32, tag="rb", name=f"rb{x}")
                nc.vector.reciprocal(rb[:, 0:ch], zt[:, 0:ch])
                row = cn + 2 * WINDOW

                def norm(c, o, w, eng):
                    dst = qs_t[n][:, c * row + WINDOW + h * ch + o:
                                  c * row + WINDOW + h * ch + o + w]
                    if eng is nc.gpsimd and not NOGP:
                        eng.scalar_tensor_tensor(
                            out=dst, in0=ebf[:, c * ch + o:c * ch + o + w],
                            scalar=1.0, in1=rb[:, o:o + w],
                            op0=mybir.AluOpType.mult,
                            op1=mybir.AluOpType.mult)
                    else:
                        nc.vector.tensor_mul(
                            out=dst, in0=ebf[:, c * ch + o:c * ch + o + w],
                            in1=rb[:, o:o + w])

                if h == 1 and M[n] > 0:
                    # junk columns [lo, ch) need the masked reciprocal; the
                    # clean bulk [0, lo) only needs recip and goes straight
                    # to the Pool so the mask never gates it.
                    lo = F[n] - ch
                    nc.vector.tensor_mul(
                        out=rb[:, lo:lo + M[n]], in0=rb[:, lo:lo + M[n]],
                        in1=mk_t[:, mk_off[n]:mk_off[n] + M[n]])
                    norm(0, 0, lo, nc.gpsimd)
                    norm(1, 0, lo, nc.gpsimd)
                    norm(0, lo, M[n], nc.vector)
                    norm(1, lo, M[n], nc.vector)
                else:
                    sp = (2 * (ch // 5)) // 8 * 8
                    norm(0, 0, sp, nc.vector)
                    norm(0, sp, ch - sp, nc.gpsimd)
                    norm(1, 0, ch, nc.gpsimd)

            def emit_conv(x):
                n, it, h = sih(x)
                ch, cn = Ch[n], C[n]
                row = cn + 2 * WINDOW
                pv = psp.tile([128, 1024], f32, tag="cv", name=f"cv{x}")
                psums[x] = pv
                qsr = qs_t[n][:, 0:NCH * row].rearrange("p (a b) -> p a b",
                                                        a=NCH)
                ucr = uc_t[n][:, 0:NCH * cn].rearrange("p (a b) -> p a b",
                                                       a=NCH)
                half = ch // 2
                blocks = [(0, half), (half, ch - half)] if not NODR \
                    else [(0, ch)]
                base = WINDOW + h * ch
                for mc in range(NCH):
                    for (co, nn) in blocks:
                        out = pv[:, mc * 512 + co:mc * 512 + co + nn]
                        first = False
                        o = mc * cn + h * ch + co
                        nc.tensor.matmul(
                            out, id_t[:], uc_t[n][:, o:o + nn],
                            start=True, stop=False)
                        if NODR:
                            nmm = len(SHIFTS) * NCH
                            cnt = 0
                            for (t, d) in SHIFTS:
                                for kt in range(NCH):
                                    lhs = w_t[:, ((t * 2 + mc) * 2 + kt) * 128:
                                              ((t * 2 + mc) * 2 + kt + 1) * 128]
                                    rhs = qs_t[n][:, kt * row + base + co + d:
                                                  kt * row + base + co + d + nn]
                                    nc.tensor.matmul(
                                        out, lhs, rhs,
                                        start=(first and cnt == 0),
                                        stop=(cnt == nmm - 1))
                                    cnt += 1
                        else:
                            for ti, (t, d) in enumerate(SHIFTS):
                                lhs = w_t[:, (t * 2 + mc) * 256:
                                          (t * 2 + mc + 1) * 256].rearrange(
                                    "p (a b) -> p a b", a=2)
                                rhs = qsr[:, :, base + co + d:
                                          base + co + d + nn]
                                nc.tensor.matmul(
                                    out, lhs, rhs,
                                    start=(first and ti == 0),
                                    stop=(ti == len(SHIFTS) - 1),
                                    perf_mode=mybir.MatmulPerfMode.DoubleRow)
            def emit_out(x):
                n, it, h = sih(x)
                if it != ITERS - 1:
                    return
                ch, cn = Ch[n], C[n]
                pv = psums[x]
                # PSUM already holds 64*(msg+ucorr) thanks to the ident
                # matmul; evacuate on the Act engine and ship.
                if True:
                    qfw = qf_t[n][:, 0:NCH * cn].rearrange(
                        "p (a b) -> p a b", a=NCH)[:, :, h * ch:(h + 1) * ch]
                    pvr = pv[:, 0:1024].rearrange(
                        "p (a b) -> p a b", a=NCH)[:, :, 0:ch]
                    nc.scalar.activation(
                        out=qfw, in_=pvr,
                        func=mybir.ActivationFunctionType.Copy)
                    qfr = qf_t[n][:, 0:NCH * cn].rearrange(
                        "p (a b) -> p a b", a=NCH)[:, :, h * ch:(h + 1) * ch]
                    qdr = q_d[n][:, 0:NCH * cn].rearrange(
                        "p (a b) -> p a b", a=NCH)[:, :, h * ch:(h + 1) * ch]
                    nc.sync.dma_start(out=qdr, in_=qfr)

            # round r emits: zchain(r), conv(r-2), exp(r+2), out(r-2).
            # (the final-iteration PSUM evacuation copy is emitted AFTER
            # exp(r+2) so it never delays the next exp in the Act queue)
            # PE queue per round = [colsum(r), conv(r-2)]: conv trails its
            # own norm by 2 rounds and the seam-partner norm by 1 round, so
            # the PE never waits; exp(r+2) is emitted right after conv(r-2)
            # (whose PSUM it reads), keeping <=3 conv PSUMs alive.
            ebfs[0] = emit_exp(0)
            ebfs[1] = emit_exp(1)
            for r in range(NSTEP + 2):
                if r == 0:
                    dma_in(2)
                if r == 2:
                    dma_in(3)
                if r < NSTEP:
                    emit_zchain(r)
                if 0 <= r - 2 < NSTEP:
                    emit_conv(r - 2)
                if r + 2 < NSTEP:
                    ebfs[r + 2] = emit_exp(r + 2)
                if 0 <= r - 2 < NSTEP:
                    emit_out(r - 2)

    _split_sync_waits(nc)
    return nc


_CACHE = {}
_LAST_NC = None


def _get_nc(plan=None):
    global _LAST_NC
    if plan is None:
        return _LAST_NC
    key = (tuple(plan["C"]), tuple(plan["F"]))
    if key not in _CACHE:
        _CACHE[key] = _build(plan["C"], plan["F"], plan["Ch"], plan["M"])
    _LAST_NC = _CACHE[key]
    return _LAST_NC


def _host_prep(unary_score, mask, transitions, start_transitions,
               end_transitions, lengths, plan):
    f16 = np.float16
    L = plan["L"]
    unary = (np.asarray(unary_score, np.float32)
             * np.asarray(mask, np.float32)[..., None])      # [B,S,T]
    trans = np.asarray(transitions, np.float32)
    ucorr = unary.copy()
    ucorr[:, 0:WINDOW, :] += np.asarray(start_transitions, np.float32)
    rowmean = [trans[j].sum(axis=1) / T for j in range(WINDOW)]  # u @ Tj^T
    endt = np.asarray(end_transitions, np.float32)
    for b in range(B):
        lb = int(L[b])
        for j in range(1, WINDOW + 1):
            ucorr[b, lb - j, :] += endt[j - 1]
        # uniform-softmax senders in the masked tail [lb, S)
        if lb <= S - 1:
            ucorr[b, lb - 1, :] += rowmean[0]      # sender lb,   j=1
            ucorr[b, lb - 2, :] += rowmean[1]      # sender lb,   j=2
        if lb <= S - 2:
            ucorr[b, lb - 1, :] += rowmean[1]      # sender lb+1, j=2
    mats = [trans[0], trans[1], trans[0].T, trans[1].T]

    qdt = ml_dtypes.bfloat16 if NODR else ml_dtypes.float8_e4m3
    wpk = np.zeros((128, 2048), np.float32)
    for t in range(4):
        for mc in range(NCH):
            for kt in range(NCH):
                blk = mats[t][kt * 128:(kt + 1) * 128,
                              mc * 128:(mc + 1) * 128]       # [kp, m]
                if NODR:
                    o = ((t * 2 + mc) * 2 + kt) * 128
                    wpk[:, o:o + 128] = blk
                else:
                    o = (t * 2 + mc) * 256 + kt * 128
                    wpk[:, o:o + 128] = blk
    wpk = wpk.astype(qdt)
    ident = np.eye(128, dtype=f16)
    onesb = np.full((128, 128), 1.0 / SCALE, ml_dtypes.bfloat16)

    u64 = (unary * SCALE).astype(f16)
    uc64 = (ucorr * SCALE).astype(f16)

    in_maps = []
    for core in range(N_CORES):
        m = {"wdr": wpk, "ident": ident, "onesb": onesb}
        for n in range(BPC):
            b = int(plan["slots"][n][core])
            cn = plan["C"][n]
            m[f"u{n}"] = np.ascontiguousarray(
                u64[b, :cn].T.reshape(NCH, 128, cn))
            m[f"uc{n}"] = np.ascontiguousarray(
                uc64[b, :cn].T.reshape(NCH, 128, cn))
            if plan["M"][n] > 0:
                lb = int(L[b])
                col = np.arange(plan["F"][n], cn) < lb
                m[f"mk{n}"] = np.ascontiguousarray(
                    np.broadcast_to(col[None, :].astype(np.float32),
                                    (128, plan["M"][n])))
        in_maps.append(m)
    return in_maps


def kernel(token_feats, unary_score, mask, transitions, start_transitions,
           end_transitions, lengths):
    plan = _plan(lengths)
    nc = _get_nc(plan)
    in_maps = _host_prep(unary_score, mask, transitions, start_transitions,
                         end_transitions, lengths, plan)
    res = bass_utils.run_bass_kernel_spmd(nc, in_maps,
                                          core_ids=list(range(N_CORES)))
    out = np.zeros((B, S, T), np.float32)
    L = plan["L"]
    for core in range(N_CORES):
        for n in range(BPC):
            b = int(plan["slots"][n][core])
            cn = plan["C"][n]
            qv = np.asarray(res.results[core][f"q{n}"],
                            np.float32)                       # [2,128,cn]
            lb = int(L[b])
            out[b, :lb, :] = qv.reshape(T, cn).T[:lb] / SCALE
    return out
